# revision 1
# baseline (speedup 1.0000x reference)
"""LocallyConnectedXYZLayer Trainium2 kernel.

out[n,c,h,w] = sum_{dy,dx in 5x5} exp(-|xyz(n,:,h+dy-2,w+dx-2)-xyz(n,:,h,w)|^2/2)
               * (softmax*mask)(n,c,h+dy-2,w+dx-2)        (zero-padded)

Sharding: 8 cores = (batch n = core//2) x (W half = core%2).
Per-core layout: partitions = 2 w-chunks x 64 h rows; free dims = (dy, c, w).
The 5 dy window shifts are baked into host-prepared per-partition rows (one
DMA per tensor per step); dx shifts are free-dim slices.  Per 5x5 offset:
one bf16 tensor_tensor mul (gaussian broadcast over channels via a step-0 AP
dim) + one accumulate add on the vector engine; squared-distance chain runs
in fp32 on gpsimd, exp on the scalar engine; per-dx group sums in bf16 with
an fp32 master accumulator (hierarchical accumulation for precision).
"""

import sys
from contextlib import ExitStack

import numpy as np

sys.path.insert(0, "/opt/trn_rl_repo")

import ml_dtypes  # noqa: E402

import concourse.bass as bass  # noqa: E402
from concourse import mybir  # noqa: E402
from concourse.bass_utils import run_bass_kernel_spmd  # noqa: E402

BF16 = ml_dtypes.bfloat16

N, C, H, W = 4, 20, 64, 2048
KH = KW = 5
PAD = 2
HH = H + 2 * PAD  # 68 padded rows
WCORE = W // 2  # 1024 interior w per core
NSTEP = 4  # device steps
WS = WCORE // (2 * NSTEP)  # 128 interior w per (step, chunk)
WX = WS + 2 * PAD  # 132 w extent (halo 2 each side)

_CACHE = {}


def _build_nc():
    """Raw-Bass program (no Tile): this toolchain's walrus codegen allows at
    most one sync-wait command per instruction, so all cross-engine sync is
    standalone wait_ge instructions plus one then_inc on producer ops."""
    nc = bass.Bass("TRN2", target_bir_lowering=False, debug=False)
    bf = mybir.dt.bfloat16
    f32 = mybir.dt.float32
    sm_d = nc.dram_tensor("sm_in", [NSTEP, 128, 2, KH, C, WX], bf,
                          kind="ExternalInput")
    xyz_d = nc.dram_tensor("xyz_in", [NSTEP, 128, KH, 3, WX], bf,
                           kind="ExternalInput")
    m_d = nc.dram_tensor("m_in", [NSTEP, 128, KH, WX], bf,
                         kind="ExternalInput")
    out_d = nc.dram_tensor("out_d", [NSTEP, 128, C, WS], f32,
                           kind="ExternalOutput")

    def sb(name, shape, dt):
        return nc.alloc_sbuf_tensor(name, list(shape), dt).ap()

    # double-buffered input tiles (per step parity)
    sm_t = [sb(f"sm{i}", [128, 2, KH, C, WX], bf) for i in range(2)]
    xyz_t = [sb(f"xyz{i}", [128, KH, 3, WX], bf) for i in range(2)]
    m_t = [sb(f"m{i}", [128, KH, WX], bf) for i in range(2)]
    # d2 chain (gpsimd-internal reuse is in-order; d2 double-buffered for ACT)
    diff_t = sb("diff", [128, KH, 3, WS], bf)
    d2_t = [sb(f"d2_{i}", [128, KH, WS], bf) for i in range(2)]
    g5_t = [sb(f"g5_{i}", [128, KH, WS], bf) for i in range(2)]
    gm5_t = sb("gm5", [128, KH, WS], bf)
    t_t = sb("t", [128, C, WS], bf)
    group_t = sb("group", [128, C, WS], bf)
    master_t = [sb(f"master{i}", [128, C, WS], f32) for i in range(2)]

    ADD, MULT, SUB = (mybir.AluOpType.add, mybir.AluOpType.mult,
                      mybir.AluOpType.subtract)

    with ExitStack() as ctx:
        load_sem = ctx.enter_context(nc.semaphore("load_sem"))
        sm_sem = ctx.enter_context(nc.semaphore("sm_sem"))
        sme_sem = ctx.enter_context(nc.semaphore("sme_sem"))
        store_sem = ctx.enter_context(nc.semaphore("store_sem"))
        pool_sem = ctx.enter_context(nc.semaphore("pool_sem"))
        act_sem = ctx.enter_context(nc.semaphore("act_sem"))
        dve_sem = ctx.enter_context(nc.semaphore("dve_sem"))
        block = ctx.enter_context(nc.Block())

        @block.sync
        def _(sync):
            for s in range(NSTEP):
                b = s % 2
                if s >= 1:
                    # DMA completions across steps are unordered; gate this
                    # step's loads on the previous step's completions so the
                    # cumulative load_sem threshold implies the right data.
                    sync.wait_ge(load_sem, 32 * s)
                    sync.wait_ge(sm_sem, 16 * s)
                    sync.wait_ge(sme_sem, 16 * s)
                if s >= 2:
                    # input buffer reuse: step s-2 consumers must be done
                    # (master of tau=5(s-2)+4; all reads precede it)
                    sync.wait_ge(dve_sem, 15 * (5 * s - 6) + 19)
                sync.dma_start(xyz_t[b][:], xyz_d[s]).then_inc(load_sem, 16)
                sync.dma_start(m_t[b][:], m_d[s]).then_inc(load_sem, 16)
                sync.dma_start(sm_t[b][:, 0], sm_d[s, :, 0]).then_inc(sme_sem, 16)
                sync.dma_start(sm_t[b][:, 1], sm_d[s, :, 1]).then_inc(sm_sem, 16)
                if s >= 1:
                    sync.wait_ge(dve_sem, 15 * (5 * s - 1) + 19)
                    if s >= 2:
                        sync.wait_ge(store_sem, 16 * (s - 1))
                    sync.dma_start(out_d[s - 1],
                                   master_t[(s - 1) % 2][:]).then_inc(
                                       store_sem, 16)
            sync.wait_ge(dve_sem, 300)
            sync.wait_ge(store_sem, 16 * (NSTEP - 1))
            sync.dma_start(out_d[NSTEP - 1],
                           master_t[(NSTEP - 1) % 2][:]).then_inc(
                               store_sem, 16)

        @block.scalar
        def _(scalar):
            for s in range(NSTEP):
                for dx in range(KW):
                    tau = 5 * s + dx
                    scalar.wait_ge(dve_sem, 4 if tau == 0 else 15 * tau - 7)
                    if tau >= 2:
                        # g5 buffer reuse: gm5 of group tau-2 must be done
                        scalar.wait_ge(dve_sem, 15 * (tau - 2) + 9)
                    scalar.activation(
                        out=g5_t[tau % 2][:], in_=d2_t[tau % 2][:],
                        func=mybir.ActivationFunctionType.Exp,
                        scale=-0.5).then_inc(act_sem)

        @block.vector
        def _(vector):
            # Everything data-parallel lives on DVE (gpsimd sharing the SBUF
            # ports slowed concurrent DVE ops ~3x, a worse trade than doing
            # the d2 chain here at 2x bf16).  The d2 chain for tau+1 is
            # emitted before the MAC of tau so ACT's exp overlaps the MAC.
            nv = [0]

            def vop(bi):
                nv[0] += 1
                return bi

            def vwait():
                if nv[0] > 0:
                    vector.wait_ge(dve_sem, nv[0])

            def d2chain(tau):
                sp, dxp = divmod(tau, KW)
                bp = sp % 2
                if dxp == 0:
                    # xyz + m suffice for the d2 chain and gm5
                    vector.wait_ge(load_sem, 32 * (sp + 1))
                if tau >= 2:
                    # d2 buffer reuse: exp of group tau-2 must be done
                    vector.wait_ge(act_sem, tau - 1)
                xyz_c = xyz_t[bp][:, 2, :, PAD:PAD + WS].unsqueeze(
                    1).broadcast_to([128, KH, 3, WS])
                vwait()
                vop(vector.tensor_tensor(
                    out=diff_t[:], in0=xyz_t[bp][:, :, :, dxp:dxp + WS],
                    in1=xyz_c, op=SUB).then_inc(dve_sem))
                vwait()
                vop(vector.tensor_tensor(
                    out=diff_t[:], in0=diff_t[:], in1=diff_t[:],
                    op=MULT).then_inc(dve_sem))
                d2 = d2_t[tau % 2]
                vwait()
                vop(vector.tensor_tensor(
                    out=d2[:], in0=diff_t[:, :, 0, :], in1=diff_t[:, :, 1, :],
                    op=ADD).then_inc(dve_sem))
                vwait()
                vop(vector.tensor_tensor(
                    out=d2[:], in0=d2[:], in1=diff_t[:, :, 2, :],
                    op=ADD).then_inc(dve_sem))

            d2chain(0)
            for tau in range(NSTEP * KW):
                s, dx = divmod(tau, KW)
                b = s % 2
                if dx == 0 and s >= 2:
                    # master buffer reuse: store of step s-2 must be done
                    vector.wait_ge(store_sem, 16 * (s - 1))
                master = master_t[b]
                if tau + 1 < NSTEP * KW:
                    d2chain(tau + 1)
                vector.wait_ge(act_sem, tau + 1)
                if dx == 0:
                    vector.wait_ge(sme_sem, 16 * (s + 1))
                elif dx == 1:
                    vector.wait_ge(sm_sem, 16 * (s + 1))
                vwait()
                vop(vector.tensor_tensor(
                    out=gm5_t[:], in0=g5_t[tau % 2][:],
                    in1=m_t[b][:, :, dx:dx + WS],
                    op=MULT).then_inc(dve_sem))
                e, off = dx % 2, dx - dx % 2
                for dy in range(KH):
                    sm_s = sm_t[b][:, e, dy, :, off:off + WS]
                    g_b = gm5_t[:, dy, :].unsqueeze(1).broadcast_to(
                        [128, C, WS])
                    if dy == 0:
                        vwait()
                        vop(vector.tensor_tensor(
                            out=group_t[:], in0=sm_s, in1=g_b,
                            op=MULT).then_inc(dve_sem))
                    else:
                        vwait()
                        vop(vector.tensor_tensor(
                            out=t_t[:], in0=sm_s, in1=g_b,
                            op=MULT).then_inc(dve_sem))
                        vwait()
                        vop(vector.tensor_tensor(
                            out=group_t[:], in0=group_t[:], in1=t_t[:],
                            op=ADD).then_inc(dve_sem))
                vwait()
                if dx == 0:
                    vop(vector.tensor_copy(
                        master[:], group_t[:]).then_inc(dve_sem))
                else:
                    vop(vector.tensor_tensor(
                        out=master[:], in0=master[:], in1=group_t[:],
                        op=ADD).then_inc(dve_sem))
            assert nv[0] == 300, nv[0]

    return nc


def _prep_core(xyz, softmax, mask, core):
    """Build the per-core dy-baked slab arrays (host side, bf16).

    Row layout: partition p (0..127) = chunk (p//64) x h row (p%64); the
    dy dim holds the 5 shifted window rows h+dy (in padded coords)."""
    n, half = core // 2, core % 2
    w0 = WCORE * half
    wp_sz = WCORE + 2 * PAD + 1
    lo, hi = w0 - PAD, w0 + WCORE + PAD + 1
    glo, ghi = max(lo, 0), min(hi, W)

    smp = np.zeros((HH, C, wp_sz), BF16)
    smp[PAD:PAD + H, :, glo - lo:ghi - lo] = (
        softmax[n][:, :, glo:ghi].transpose(1, 0, 2).astype(BF16))
    xyzp = np.zeros((HH, 3, wp_sz), BF16)
    xyzp[PAD:PAD + H, :, glo - lo:ghi - lo] = (
        xyz[n][:, :, glo:ghi].transpose(1, 0, 2).astype(BF16))
    mp = np.zeros((HH, wp_sz), BF16)
    mp[PAD:PAD + H, glo - lo:ghi - lo] = mask[n][:, glo:ghi].astype(BF16)

    sm5 = np.empty((NSTEP, 128, 2, KH, C, WX), BF16)
    xyz5 = np.empty((NSTEP, 128, KH, 3, WX), BF16)
    m5 = np.empty((NSTEP, 128, KH, WX), BF16)
    for s in range(NSTEP):
        for chunk in range(2):
            wb = WS * (s + NSTEP * chunk)
            pr = slice(64 * chunk, 64 * chunk + 64)
            for dy in range(KH):
                for e in range(2):
                    sm5[s, pr, e, dy] = smp[dy:dy + 64, :, wb + e:wb + e + WX]
                xyz5[s, pr, dy] = xyzp[dy:dy + 64, :, wb:wb + WX]
                m5[s, pr, dy] = mp[dy:dy + 64, wb:wb + WX]
    return {"sm_in": sm5, "xyz_in": xyz5, "m_in": m5}


def make_in_maps(xyz, softmax, mask):
    return [_prep_core(xyz, softmax, mask, k) for k in range(8)]


def assemble_out(results):
    out = np.empty((N, C, H, W), np.float32)
    for core in range(8):
        n, half = core // 2, core % 2
        w0 = WCORE * half
        o = np.asarray(results[core]["out_d"], dtype=np.float32)
        # [s, chunk*64+h, c, j] -> [c, h, (s + NSTEP*chunk)*WS + j]
        o = o.reshape(NSTEP, 2, H, C, WS)
        # -> [c, h, chunk, s, j]
        out[n, :, :, w0:w0 + WCORE] = o.transpose(3, 2, 1, 0, 4).reshape(
            C, H, WCORE)
    return out


def get_nc():
    if "nc" not in _CACHE:
        _CACHE["nc"] = _build_nc()
    return _CACHE["nc"]


def kernel(xyz, softmax, mask, trace=False, trace_kwargs=None):
    nc = get_nc()
    in_maps = make_in_maps(np.asarray(xyz), np.asarray(softmax),
                           np.asarray(mask))
    res = run_bass_kernel_spmd(nc, in_maps, list(range(8)), trace=trace,
                               **(trace_kwargs or {}))
    out = assemble_out(res.results)
    if trace:
        return out, res
    return out



# revision 8
# speedup vs baseline: 2.0757x; 2.0757x over previous
"""LocallyConnectedXYZLayer Trainium2 kernel (v2).

out[n,c,h,w] = sum_{dy,dx in 5x5} exp(-|xyz(n,:,h+dy-2,w+dx-2)-xyz(n,:,h,w)|^2/2)
               * (softmax*mask)(n,c,h+dy-2,w+dx-2)        (zero-padded)

Sharding: 8 cores = (batch n = core//2) x (W half = core%2).
Per-core layout: partitions = 2 w-chunks x 64 h rows; free dims carry
(dy, dx, c, w).  dy window shifts are host-baked into per-partition rows;
dx shifts are free-dim slices, duplicated at +0/+1 (parity e) so every
strided slice stays 4-byte aligned and the DVE runs tensor_tensor in
2x bf16 mode throughout.

Engine split per step (8 steps of 64 interior w per chunk):
  DVE   : diff = xyz_shift - xyz_center (2 merged subs), d2 = sum of
          squares (2 adds), t = g * sm (10 merged muls, c broadcast via
          stride-0 dim).
  ACT   : Square(diff), Exp(-d2/2), PSUM->SBUF output drains (all three
          live in the exp_and_others table set -> one table load).
  PE    : accumulates the 25 shifted products into PSUM with
          matmul(Identity, t_k) -- the adds cost nothing on DVE; the
          center term (g==1) is accumulated straight from sm, skipping
          its multiply.  fp32 PSUM accumulation, bf16 output store.
mask is folded into softmax on the host.
"""

import sys
from contextlib import ExitStack

import numpy as np

sys.path.insert(0, "/opt/trn_rl_repo")

import ml_dtypes  # noqa: E402

import concourse.bass as bass  # noqa: E402
from concourse import mybir  # noqa: E402
from concourse.bass_utils import run_bass_kernel_spmd  # noqa: E402

BF16 = ml_dtypes.bfloat16

N, C, H, W = 4, 20, 64, 2048
KH = KW = 5
PAD = 2
HH = H + 2 * PAD  # 68 padded rows
WCORE = W // 2  # 1024 interior w per core
NSTEP = 8
WS = WCORE // (2 * NSTEP)  # 64 interior w per (step, chunk)
WX = WS + 2 * PAD  # 68 w extent (halo 2 each side)

# per-partition element pitches
XP = 2 * KH * 3 * WX          # xyz tile pitch (2040)
SP = 2 * KH * C * WX          # sm tile pitch (27200)
SE = KH * C * WX              # sm parity block (6800)
DP = KH * 3 * KW * WS         # diff pitch (4800)
GP = KH * KW * WS             # d2/g pitch (1600)
TP = KW * C * WS              # t pitch (6400)
OP = C * WS                   # out pitch (1280)

_CACHE = {}


def _build_nc():
    """Raw-Bass program; cross-engine sync is standalone wait_ge
    instructions plus one then_inc per producer op (walrus allows at most
    one sync command per instruction)."""
    nc = bass.Bass("TRN2", target_bir_lowering=False, debug=False)
    bf = mybir.dt.bfloat16
    f32 = mybir.dt.float32
    sm_d = nc.dram_tensor("sm_in", [NSTEP, 128, 2, KH, C, WX], bf,
                          kind="ExternalInput")
    xyz_d = nc.dram_tensor("xyz_in", [NSTEP, 128, 2, KH, 3, WX], bf,
                           kind="ExternalInput")
    id_d = nc.dram_tensor("ident_in", [128, 128], bf, kind="ExternalInput")
    out_d = nc.dram_tensor("out_d", [NSTEP, 128, C, WS], bf,
                           kind="ExternalOutput")

    def sb(name, shape, dt):
        return nc.alloc_sbuf_tensor(name, list(shape), dt)

    xyz_t = [sb(f"xyz{i}", [128, 2, KH, 3, WX], bf) for i in range(2)]
    sm_t = [sb(f"sm{i}", [128, 2, KH, C, WX], bf) for i in range(2)]
    diff_t = sb("diff", [128, KH, 3, KW, WS], bf)
    sq_t = sb("sq", [128, KH, 3, KW, WS], bf)
    d2_t = [sb(f"d2_{i}", [128, KH, KW, WS], bf) for i in range(2)]
    g5_t = [sb(f"g5_{i}", [128, KH, KW, WS], bf) for i in range(2)]
    t_t = [sb(f"t{i}", [128, KW, C, WS], bf) for i in range(2)]
    out_t = [sb(f"out{i}", [128, C, WS], bf) for i in range(2)]
    id_t = sb("ident", [128, 128], bf)
    ps_t = [nc.alloc_psum_tensor(f"ps{i}", [128, OP], f32) for i in range(2)]

    ADD, MULT, SUB = (mybir.AluOpType.add, mybir.AluOpType.mult,
                      mybir.AluOpType.subtract)

    # column tiles for the PE/PSUM accumulation (c,w flattened)
    CT = [(0, 512), (512, 512), (1024, 256)]
    # dx slots per (dy, parity): even slots from parity-0 data, odd from
    # parity-1; dy==2 drops the center (dx==2) from the even list.
    def dxs_even(dy):
        return (0, 4) if dy == 2 else (0, 2, 4)

    # DVE ops per step block: 2 d2adds, 10 muls, 6 subs (for s+1).
    # Block positions: d2a 1-2, muls dy0 3-4, dy1 5-6, subs 7-12,
    # dy2 13-14, dy3 15-16, dy4 17-18.  Prologue: 6 subs.
    DVE_STEP = 18
    DVE_PRO = 6

    def dve_at(s, pos):
        # semaphore value after `pos` ops of step-s block
        return DVE_PRO + DVE_STEP * s + pos

    with ExitStack() as ctx:
        load_sem = ctx.enter_context(nc.semaphore("load_sem"))
        sm0_sem = ctx.enter_context(nc.semaphore("sm0_sem"))
        sm1_sem = ctx.enter_context(nc.semaphore("sm1_sem"))
        id_sem = ctx.enter_context(nc.semaphore("id_sem"))
        store_sem = ctx.enter_context(nc.semaphore("store_sem"))
        dve_sem = ctx.enter_context(nc.semaphore("dve_sem"))
        act_sem = ctx.enter_context(nc.semaphore("act_sem"))
        drain_sem = ctx.enter_context(nc.semaphore("drain_sem"))
        pe_sem = ctx.enter_context(nc.semaphore("pe_sem"))
        block = ctx.enter_context(nc.Block())

        @block.sync
        def _(sync):
            sync.dma_start(id_t.ap(), id_d[:]).then_inc(id_sem, 16)
            for s in range(NSTEP):
                if s >= 1:
                    # DMA completions across steps are unordered; gate on
                    # the previous step's completions so cumulative
                    # thresholds imply the right data.
                    sync.wait_ge(load_sem, 16 * s)
                    sync.wait_ge(sm0_sem, 16 * s)
                    sync.wait_ge(sm1_sem, 16 * s)
                if s >= 2:
                    # tile reuse: step s-2 consumers must be done
                    sync.wait_ge(dve_sem, dve_at(s - 1, 0))
                    sync.wait_ge(pe_sem, 75 * (s - 1))
                b = s % 2
                sync.dma_start(xyz_t[b].ap(), xyz_d[s]).then_inc(load_sem, 16)
                sync.dma_start(sm_t[b][:, 0], sm_d[s, :, 0]).then_inc(
                    sm0_sem, 16)
                sync.dma_start(sm_t[b][:, 1], sm_d[s, :, 1]).then_inc(
                    sm1_sem, 16)
                if s >= 1:
                    sync.wait_ge(drain_sem, 3 * s)
                    sync.wait_ge(store_sem, 16 * (s - 1))
                    sync.dma_start(out_d[s - 1],
                                   out_t[(s - 1) % 2].ap()).then_inc(
                                       store_sem, 16)
            sync.wait_ge(drain_sem, 3 * NSTEP)
            sync.wait_ge(store_sem, 16 * (NSTEP - 1))
            sync.dma_start(out_d[NSTEP - 1],
                           out_t[(NSTEP - 1) % 2].ap()).then_inc(
                               store_sem, 16)

        @block.vector
        def _(vector):
            def subs(k, small=False):
                # diff[dy,i,dx,w] = xyz[e][dy,i,dx',w] - xyz[e0][2,i,2+w]
                # ISA allows 3 free dims -> one instruction per component i
                kh, ws = (1, 2) if small else (KH, WS)
                xt = xyz_t[k % 2]
                for i in range(3):
                    cen3 = bass.AP(xt, (2 * 3 + i) * WX + PAD,
                                   [[XP, 128], [0, kh], [0, 3], [1, ws]])
                    cen2 = bass.AP(xt, (2 * 3 + i) * WX + PAD,
                                   [[XP, 128], [0, kh], [0, 2], [1, ws]])
                    in_e = bass.AP(xt, i * WX,
                                   [[XP, 128], [3 * WX, kh], [2, 3], [1, ws]])
                    out_e = bass.AP(diff_t, i * KW * WS,
                                    [[DP, 128], [3 * KW * WS, kh],
                                     [2 * WS, 3], [1, ws]])
                    vector.tensor_tensor(out=out_e, in0=in_e, in1=cen3,
                                         op=SUB).then_inc(dve_sem)
                    in_o = bass.AP(xt, KH * 3 * WX + i * WX,
                                   [[XP, 128], [3 * WX, kh], [2, 2], [1, ws]])
                    out_o = bass.AP(diff_t, i * KW * WS + WS,
                                    [[DP, 128], [3 * KW * WS, kh],
                                     [2 * WS, 2], [1, ws]])
                    vector.tensor_tensor(out=out_o, in0=in_o, in1=cen2,
                                         op=SUB).then_inc(dve_sem)

            def d2adds(s):
                d2 = bass.AP(d2_t[s % 2], 0,
                             [[GP, 128], [KW * WS, KH], [WS, KW], [1, WS]])
                sq_i = [bass.AP(sq_t, i * KW * WS,
                                [[DP, 128], [3 * KW * WS, KH], [WS, KW],
                                 [1, WS]]) for i in range(3)]
                vector.tensor_tensor(out=d2, in0=sq_i[0], in1=sq_i[1],
                                     op=ADD).then_inc(dve_sem)
                vector.tensor_tensor(out=d2, in0=d2, in1=sq_i[2],
                                     op=ADD).then_inc(dve_sem)

            def muls(s, dy):
                st, g5, tt = sm_t[s % 2], g5_t[s % 2], t_t[dy % 2]
                de = dxs_even(dy)
                stride = de[1] - de[0]
                out_e = bass.AP(tt, de[0] * C * WS,
                                [[TP, 128], [stride * C * WS, len(de)],
                                 [WS, C], [1, WS]])
                sm_e = bass.AP(st, dy * C * WX,
                               [[SP, 128], [stride, len(de)], [WX, C],
                                [1, WS]])
                g_e = bass.AP(g5, dy * KW * WS + de[0] * WS,
                              [[GP, 128], [stride * WS, len(de)], [0, C],
                               [1, WS]])
                vector.tensor_tensor(out=out_e, in0=sm_e, in1=g_e,
                                     op=MULT).then_inc(dve_sem)
                out_o = bass.AP(tt, C * WS,
                                [[TP, 128], [2 * C * WS, 2], [WS, C],
                                 [1, WS]])
                sm_o = bass.AP(st, SE + dy * C * WX,
                               [[SP, 128], [2, 2], [WX, C], [1, WS]])
                g_o = bass.AP(g5, dy * KW * WS + WS,
                              [[GP, 128], [2 * WS, 2], [0, C], [1, WS]])
                vector.tensor_tensor(out=out_o, in0=sm_o, in1=g_o,
                                     op=MULT).then_inc(dve_sem)

            vector.wait_ge(load_sem, 16)
            subs(0)
            for s in range(NSTEP):
                vector.wait_ge(act_sem, 2 * s + 1)
                d2adds(s)
                vector.wait_ge(act_sem, 2 * s + 2)
                vector.wait_ge(sm0_sem, 16 * (s + 1))
                vector.wait_ge(sm1_sem, 16 * (s + 1))
                if s >= 1:
                    vector.wait_ge(pe_sem, 75 * s)
                muls(s, 0)
                if s >= 1:
                    vector.wait_ge(pe_sem, 75 * s - 15)
                muls(s, 1)
                if s + 1 < NSTEP:
                    vector.wait_ge(load_sem, 16 * (s + 2))
                    subs(s + 1)
                else:
                    # keep DVE_STEP uniform: two tiny dummy subs
                    subs(s, small=True)
                vector.wait_ge(pe_sem, 75 * s + 15)
                muls(s, 2)
                vector.wait_ge(pe_sem, 75 * s + 30)
                muls(s, 3)
                vector.wait_ge(pe_sem, 75 * s + 45)
                muls(s, 4)

        @block.scalar
        def _(scalar):
            EXP = mybir.ActivationFunctionType.Exp
            SQR = mybir.ActivationFunctionType.Square

            def sq(s):
                scalar.wait_ge(dve_sem, dve_at(s - 1, 12) if s else DVE_PRO)
                scalar.activation(
                    out=bass.AP(sq_t, 0, [[DP, 128], [1, DP]]),
                    in_=bass.AP(diff_t, 0, [[DP, 128], [1, DP]]),
                    func=SQR).then_inc(act_sem)

            def exp(s):
                scalar.wait_ge(dve_sem, dve_at(s, 2))
                scalar.activation(
                    out=bass.AP(g5_t[s % 2], 0, [[GP, 128], [1, GP]]),
                    in_=bass.AP(d2_t[s % 2], 0, [[GP, 128], [1, GP]]),
                    func=EXP, scale=-0.5).then_inc(act_sem)

            sq(0)
            exp(0)
            for s in range(NSTEP):
                if s + 1 < NSTEP:
                    sq(s + 1)
                    exp(s + 1)
                scalar.wait_ge(pe_sem, 75 * (s + 1))
                if s >= 2:
                    scalar.wait_ge(store_sem, 16 * (s - 1))
                for lo, ln in CT:
                    scalar.activation(
                        out=bass.AP(out_t[s % 2], lo, [[OP, 128], [1, ln]]),
                        in_=ps_t[s % 2].ap()[:, lo:lo + ln],
                        func=mybir.ActivationFunctionType.Copy).then_inc(
                            drain_sem)

        @block.tensor
        def _(tensor):
            tensor.wait_ge(id_sem, 16)
            lhsT = id_t.ap()
            for s in range(NSTEP):
                ps = ps_t[s % 2]
                for dy in range(KH):
                    tensor.wait_ge(dve_sem, dve_at(s, (4, 6, 14, 16, 18)[dy]))
                    if dy == 0 and s >= 1:
                        tensor.wait_ge(drain_sem, 3 * (s - 1))
                    if dy == 2:
                        tensor.wait_ge(sm0_sem, 16 * (s + 1))
                    slots = (0, 1, 3, 4) if dy == 2 else range(KW)
                    for dxs in slots:
                        for lo, ln in CT:
                            rhs = bass.AP(t_t[dy % 2], dxs * C * WS + lo,
                                          [[TP, 128], [1, ln]])
                            tensor.matmul(
                                ps.ap()[:, lo:lo + ln], lhsT, rhs,
                                start=(dy == 0 and dxs == 0),
                                stop=(dy == 4 and dxs == 4),
                                skip_group_check=True).then_inc(pe_sem)
                    if dy == 2:
                        # center term: g == 1, accumulate sm directly
                        for lo, ln in CT:
                            c0, ncl = lo // WS, ln // WS
                            rhs = bass.AP(sm_t[s % 2],
                                          2 * C * WX + c0 * WX + PAD,
                                          [[SP, 128], [WX, ncl], [1, WS]])
                            tensor.matmul(
                                ps.ap()[:, lo:lo + ln], lhsT, rhs,
                                start=False, stop=False,
                                skip_group_check=True).then_inc(pe_sem)

    return nc


def _prep_core(xyz, softmax, mask, core):
    """Host-side slab bake (bf16): fold mask into softmax, pad, and lay
    out dy-shifted parity-duplicated windows per partition row."""
    n, half = core // 2, core % 2
    w0 = WCORE * half
    wp_sz = WCORE + 2 * PAD + 1
    lo, hi = w0 - PAD, w0 + WCORE + PAD + 1
    glo, ghi = max(lo, 0), min(hi, W)

    sm_m = (softmax[n][:, :, glo:ghi] *
            mask[n][None, :, glo:ghi]).astype(BF16)
    smp = np.zeros((HH, C, wp_sz), BF16)
    smp[PAD:PAD + H, :, glo - lo:ghi - lo] = sm_m.transpose(1, 0, 2)
    xyzp = np.zeros((HH, 3, wp_sz), BF16)
    xyzp[PAD:PAD + H, :, glo - lo:ghi - lo] = (
        xyz[n][:, :, glo:ghi].transpose(1, 0, 2).astype(BF16))

    sm5 = np.empty((NSTEP, 128, 2, KH, C, WX), BF16)
    xyz5 = np.empty((NSTEP, 128, 2, KH, 3, WX), BF16)
    for s in range(NSTEP):
        for chunk in range(2):
            wb = WS * (s + NSTEP * chunk)
            pr = slice(64 * chunk, 64 * chunk + 64)
            for dy in range(KH):
                for e in range(2):
                    sm5[s, pr, e, dy] = smp[dy:dy + 64, :, wb + e:wb + e + WX]
                    xyz5[s, pr, e, dy] = xyzp[dy:dy + 64, :,
                                              wb + e:wb + e + WX]
    ident = np.eye(128, dtype=BF16)
    return {"sm_in": sm5, "xyz_in": xyz5, "ident_in": ident}


def make_in_maps(xyz, softmax, mask):
    return [_prep_core(xyz, softmax, mask, k) for k in range(8)]


def assemble_out(results):
    out = np.empty((N, C, H, W), np.float32)
    for core in range(8):
        n, half = core // 2, core % 2
        w0 = WCORE * half
        o = np.asarray(results[core]["out_d"]).astype(np.float32)
        # [s, chunk*64+h, c, j] -> [c, h, chunk*512 + s*64 + j]
        o = o.reshape(NSTEP, 2, H, C, WS)
        out[n, :, :, w0:w0 + WCORE] = o.transpose(3, 2, 1, 0, 4).reshape(
            C, H, WCORE)
    return out


def get_nc():
    if "nc" not in _CACHE:
        _CACHE["nc"] = _build_nc()
    return _CACHE["nc"]


def kernel(xyz, softmax, mask, trace=False, trace_kwargs=None):
    nc = get_nc()
    in_maps = make_in_maps(np.asarray(xyz), np.asarray(softmax),
                           np.asarray(mask))
    res = run_bass_kernel_spmd(nc, in_maps, list(range(8)), trace=trace,
                               **(trace_kwargs or {}))
    out = assemble_out(res.results)
    if trace:
        return out, res
    return out
